# revision 1
# baseline (speedup 1.0000x reference)
"""Causal self-attention (GPT-style block) on 8 Trainium2 NeuronCores.

Sharding: pure data-parallel over batch. B=8 batch elements map 1:1 onto the
8 cores; every core runs the full per-sequence attention, so no collectives
are needed and the load is perfectly balanced.

Host-side prep (inside kernel(), before dispatch): x, w_attn, w_proj are
transposed and cast to bf16 on the host, so the device program receives
x^T [C,T], w_attn^T [C,3C], w_proj^T [C,C] with the contraction dim already
on partitions — no on-device input transposes.

Per-core device program (T=1024, C=768, H=12, hd=64):
  1. qkv from x^T/w^T in bf16 (fp32 PSUM): q^T,k^T land as [o,t] chunks
     (a head PAIR per 128-partition chunk); v lands natural [t,o] augmented
     with a ones column per head for fused softmax sums.
  2. Per head: S^T = k @ q^T (both heads of a chunk run concurrently via
     PE row-tiling, K=64 each). exp() on ScalarE with the 1/sqrt(hd) scale
     folded in; no max-subtraction (scores are O(1) for this problem's
     input distribution; fp32 exp cannot overflow). Causality by skipping
     fully-masked chunk pairs plus one triangular mask-multiply on the
     diagonal 128x128 block.
  3. y = P @ v with expS^T slices as the stationary operand in bf16:
     out[tq, 64+1] accumulates over tk chunks; column 64 is the softmax
     denominator (from the ones column). Normalization is a per-partition
     reciprocal + tensor_scalar multiply.
  4. y (bf16) is transposed via the DMA xbar and projected against
     w_proj^T in bf16; bias + output drain in fp32.
"""

import sys
from contextlib import ExitStack

import numpy as np

if "/opt/trn_rl_repo" not in sys.path:
    sys.path.insert(0, "/opt/trn_rl_repo")

import concourse.bacc as bacc
import concourse.bass as bass
import concourse.tile as tile
from concourse import mybir
from concourse.masks import make_upper_triangular

F32 = mybir.dt.float32
BF16 = mybir.dt.bfloat16

T = 1024
C = 768
H = 12
HD = C // H  # 64
N_CORES = 8


def build_attention_core(t=T, repeats=1):
    """Build the single-core Bass program (SPMD across 8 cores).

    repeats>1 emits the whole computation that many times into one NEFF —
    used only for benchmarking (amortizes host dispatch overhead).
    """
    nc = bacc.Bacc(None, target_bir_lowering=False, debug=False)
    xT_d = nc.declare_dram_parameter("xT", [C, t], BF16, isOutput=False)
    waT_d = nc.declare_dram_parameter("waT", [C, 3 * C], BF16, isOutput=False)
    b_attn = nc.declare_dram_parameter("b_attn", [3 * C], F32, isOutput=False)
    wpT_d = nc.declare_dram_parameter("wpT", [C, C], BF16, isOutput=False)
    b_proj = nc.declare_dram_parameter("b_proj", [C], F32, isOutput=False)
    out = nc.declare_dram_parameter("out", [t, C], F32, isOutput=True)

    with ExitStack() as octx:
        tc = octx.enter_context(tile.TileContext(nc))
        for _rep in range(repeats):
            _emit_once(nc, tc, t, xT_d, waT_d, b_attn, wpT_d, b_proj, out)
    nc.compile()
    return nc


def _emit_once(nc, tc, t, xT_d, waT_d, b_attn, wpT_d, b_proj, out):
    NT = t // 128  # t-chunks
    NCC = C // 128  # c-chunks (6)
    NHP = H // 2  # head pairs (6)

    with ExitStack() as ctx:
        singles = ctx.enter_context(tc.tile_pool(name="singles", bufs=1))
        psum = ctx.enter_context(tc.tile_pool(name="psum", bufs=1, space="PSUM"))

        # ---- constants -------------------------------------------------
        # keep-mask for the diagonal S^T block: 1.0 where tk(part) <= tq(col)
        tri = singles.tile([128, 128], BF16)
        make_upper_triangular(nc, tri, val=1.0, diag=True)

        # b_attn[0:2*C] rearranged so column j holds the per-partition bias
        # of qk o-chunk j ([128,1] slices for tensor_scalar_add).
        bias_qk = singles.tile([128, 2 * NCC], F32)
        nc.sync.dma_start(
            out=bias_qk,
            in_=b_attn[0 : 2 * C].rearrange("(c p) -> p c", p=128),
        )
        # v bias broadcast along partitions: [128, C]
        bias_v = singles.tile([128, C], F32)
        bav = b_attn[2 * C : 3 * C].rearrange("(o c) -> o c", o=1)
        nc.gpsimd.dma_start(
            out=bias_v,
            in_=bass.AP(tensor=bav.tensor, offset=bav.offset, ap=[[0, 128]] + bav.ap[1:]),
        )
        bias_p = singles.tile([128, C], F32)
        bpv = b_proj[:].rearrange("(o c) -> o c", o=1)
        nc.gpsimd.dma_start(
            out=bias_p,
            in_=bass.AP(tensor=bpv.tensor, offset=bpv.offset, ap=[[0, 128]] + bpv.ap[1:]),
        )

        # w_proj^T: needed only in phase E; load on the Pool (SWDGE) queue so
        # it overlaps earlier phases without blocking SP or ACT.
        wpT = singles.tile([128, NCC, C], BF16, name="wpT")
        for cc in range(NCC):
            nc.gpsimd.dma_start(
                out=wpT[:, cc, :], in_=wpT_d[cc * 128 : (cc + 1) * 128, :]
            )

        def n_pieces(total, maxw=512):
            res = []
            s = 0
            while s < total:
                w = min(maxw, total - s)
                res.append((s, w))
                s += w
            return res

        # Pools are stack-allocated in entry order and close LIFO, nested by
        # actual tensor lifetime:
        #   pool_y  (y_nat):          phases B..D
        #   pool_qkv (qT/kT/v_aug):   phases B..C   (closes before D)
        #     pool1 (xT/waT loads):   phases A..B
        #     pool_att (expS, rcp):   phase C
        #   pool_de (yT/out):         phases D..E   (reuses pool_qkv space)
        pool_y = ctx.enter_context(tc.tile_pool(name="pool_y", bufs=1))
        y_nat = [pool_y.tile([128, C], BF16, name=f"ynat{j}") for j in range(NT)]

        pool2_cm = tc.tile_pool(name="pool_qkv", bufs=1)
        pool2 = pool2_cm.__enter__()

        qT = [pool2.tile([128, t], BF16, name=f"qT{j}") for j in range(NHP)]
        kT = [pool2.tile([128, t], BF16, name=f"kT{j}") for j in range(NHP)]
        # v augmented with a ones column per head: [128, H, HD+1] per t-chunk
        v_aug = [pool2.tile([128, H, HD + 1], BF16, name=f"vaug{i}") for i in range(NT)]

        # ================= phase A+B: load + qkv =======================
        with tc.tile_pool(name="pool1", bufs=1) as pool1:
            # x^T chunks [c-part, t-free], direct load (pre-transposed on host)
            xTall = pool1.tile([128, NCC, t], BF16, name="xTall")
            for cc in range(NCC):
                nc.sync.dma_start(
                    out=xTall[:, cc, :], in_=xT_d[cc * 128 : (cc + 1) * 128, :]
                )

            for i in range(NT):
                nc.vector.memset(v_aug[i][:, :, HD : HD + 1], 1.0)

            # waT on the ACT HWDGE queue so it streams concurrently with
            # the xT loads on SP.
            waT = pool1.tile([128, NCC, 3 * C], BF16, name="waT")
            for half in range(4):
                o0, o1 = half * 3 * C // 4, (half + 1) * 3 * C // 4
                for cc in range(NCC):
                    nc.scalar.dma_start(
                        out=waT[:, cc, o0:o1],
                        in_=waT_d[cc * 128 : (cc + 1) * 128, o0:o1],
                    )

            def emit_qkv_pair(p):
                """q (og=p), k (og=6+p), v (og=12+p) chunks for pair p."""
                for og in (p, NCC + p):
                    for (s, w) in n_pieces(t):
                        pq = psum.tile([128, 512], F32, name="ps_mm", tag="ps_mm", bufs=5)
                        for cc in range(NCC):
                            nc.tensor.matmul(
                                pq[:, :w],
                                waT[:, cc, og * 128 : (og + 1) * 128],
                                xTall[:, cc, s : s + w],
                                start=(cc == 0),
                                stop=(cc == NCC - 1),
                            )
                        dst = qT[og] if og < NCC else kT[og - NCC]
                        nc.vector.tensor_scalar_add(
                            dst[:, s : s + w], pq[:, :w], bias_qk[:, og : og + 1]
                        )
                og = 2 * NCC + p
                for it in range(NT):
                    pv = psum.tile([128, 128], F32, name="ps_v", tag="ps_v", bufs=1)
                    for cc in range(NCC):
                        nc.tensor.matmul(
                            pv,
                            xTall[:, cc, it * 128 : (it + 1) * 128],
                            waT[:, cc, og * 128 : (og + 1) * 128],
                            start=(cc == 0),
                            stop=(cc == NCC - 1),
                        )
                    nc.vector.tensor_add(
                        v_aug[it][:, 2 * p : 2 * p + 2, 0:HD],
                        pv.rearrange("p (h d) -> p h d", d=HD),
                        bias_v[:, 128 * p : 128 * (p + 1)].rearrange(
                            "p (h d) -> p h d", d=HD
                        ),
                    )

            # ===== phase C: attention (expS lives in pool2, which already
            # spans phases B..C; emission is software-pipelined so pair p's
            # attention overlaps pair p+1's qkv on the PE) =====
            def emit_attention_qk(hp):
                hA, hB = 2 * hp, 2 * hp + 1
                eA = [
                    pool2.tile([128, t], BF16, name=f"eA{i}", tag=f"eA{i}", bufs=2)
                    for i in range(NT)
                ]
                eB = [
                    pool2.tile([128, t], BF16, name=f"eB{i}", tag=f"eB{i}", bufs=2)
                    for i in range(NT)
                ]
                for i in range(NT):
                    # S^T chunk: out[tk 128i.., tq 128i..t); both heads run
                    # concurrently via PE row-tiling (K=64 at 0-63 / 64-127).
                    for (s, w) in n_pieces(t - 128 * i):
                        tq0 = 128 * i + s
                        for head, half, e in ((hA, 0, eA), (hB, 64, eB)):
                            ps = psum.tile(
                                [128, 512], F32, name="ps_s", tag="ps_mm", bufs=5
                            )
                            nc.tensor.matmul(
                                ps[:, :w],
                                kT[hp][half : half + 64, 128 * i : 128 * (i + 1)],
                                qT[hp][half : half + 64, tq0 : tq0 + w],
                                start=True,
                                stop=True,
                            )
                            nc.scalar.activation(
                                e[i][:, tq0 : tq0 + w],
                                ps[:, :w],
                                mybir.ActivationFunctionType.Exp,
                                bias=0.0,
                                scale=1.0 / float(np.sqrt(HD)),
                            )
                    # causal mask on the diagonal block (keep tk <= tq)
                    d0 = 128 * i
                    nc.vector.tensor_mul(
                        eA[i][:, d0 : d0 + 128], eA[i][:, d0 : d0 + 128], tri
                    )
                    nc.vector.tensor_mul(
                        eB[i][:, d0 : d0 + 128], eB[i][:, d0 : d0 + 128], tri
                    )

                return eA, eB

            def emit_attention_pv(hp, eA, eB):
                hA, hB = 2 * hp, 2 * hp + 1
                # PV: for each tq chunk j accumulate over tk chunks i<=j.
                for head, e in ((hA, eA), (hB, eB)):
                    for j in range(NT):
                        py = psum.tile([128, HD + 1], F32, name="ps_y", tag="ps_y", bufs=2)
                        for i in range(j + 1):
                            nc.tensor.matmul(
                                py,
                                e[i][:, 128 * j : 128 * (j + 1)],
                                v_aug[i][:, head, :],
                                start=(i == 0),
                                stop=(i == j),
                            )
                        rcp = pool2.tile([128, 1], F32, name="rcp", tag="rcp", bufs=4)
                        nc.vector.reciprocal(rcp, py[:, HD : HD + 1])
                        nc.vector.tensor_scalar_mul(
                            y_nat[j][:, head * HD : (head + 1) * HD], py[:, 0:HD], rcp
                        )

            # two-stage stagger: while pair p's qkv runs, pair p-1 does
            # QK+exp and pair p-2 does PV.
            es = {}
            emit_qkv_pair(0)
            emit_qkv_pair(1)
            es[0] = emit_attention_qk(0)
            for p in range(2, NHP):
                emit_qkv_pair(p)
                es[p - 1] = emit_attention_qk(p - 1)
                emit_attention_pv(p - 2, *es.pop(p - 2))
            es[NHP - 1] = emit_attention_qk(NHP - 1)
            emit_attention_pv(NHP - 2, *es.pop(NHP - 2))
            emit_attention_pv(NHP - 1, *es.pop(NHP - 1))

        pool2_cm.__exit__(None, None, None)

        # ================= phase D+E: transpose y, project =============
        pool4 = ctx.enter_context(tc.tile_pool(name="pool_de", bufs=1))
        yTall = pool4.tile([128, NCC, t], BF16, name="yTall")
        for j in range(NT):
            nc.sync.dma_start_transpose(
                yTall[:, :, j * 128 : (j + 1) * 128], y_nat[j]
            )

        for it in range(NT):
            out_sb = pool4.tile([128, C], F32, name="out_sb", bufs=3)
            for (s, w) in n_pieces(C):
                po = psum.tile([128, 512], F32, name="ps_o", tag="ps_mm", bufs=5)
                for cc in range(NCC):
                    nc.tensor.matmul(
                        po[:, :w],
                        yTall[:, cc, it * 128 : (it + 1) * 128],
                        wpT[:, cc, s : s + w],
                        start=(cc == 0),
                        stop=(cc == NCC - 1),
                    )
                nc.vector.tensor_add(
                    out_sb[:, s : s + w], po[:, :w], bias_p[:, s : s + w]
                )
            eng = nc.gpsimd if it % 2 == 0 else nc.sync
            eng.dma_start(out=out[it * 128 : (it + 1) * 128, :], in_=out_sb)


_NC_CACHE = {}


def get_nc(t=T):
    if t not in _NC_CACHE:
        _NC_CACHE[t] = build_attention_core(t)
    return _NC_CACHE[t]


def _to_bf16(a):
    import ml_dtypes

    return np.ascontiguousarray(np.asarray(a, dtype=np.float32)).astype(
        ml_dtypes.bfloat16
    )


def host_prep(inputs):
    """Transpose + cast weights/x on the host for the device program."""
    x = np.asarray(inputs["x"], dtype=np.float32)
    b_attn = np.ascontiguousarray(inputs["b_attn"], dtype=np.float32)
    b_proj = np.ascontiguousarray(inputs["b_proj"], dtype=np.float32)
    waT = _to_bf16(np.asarray(inputs["w_attn"], dtype=np.float32).T)  # [C, 3C]
    wpT = _to_bf16(np.asarray(inputs["w_proj"], dtype=np.float32).T)  # [C, C]
    return [
        {
            "xT": _to_bf16(x[b].T),  # [C, T]
            "waT": waT,
            "b_attn": b_attn,
            "wpT": wpT,
            "b_proj": b_proj,
        }
        for b in range(x.shape[0])
    ]


def kernel(**inputs):
    from concourse.bass_utils import run_bass_kernel_spmd

    x = inputs["x"]
    B, t, _ = x.shape
    assert B == N_CORES
    in_maps = host_prep(inputs)
    nc = get_nc(t)
    res = run_bass_kernel_spmd(nc, in_maps, core_ids=list(range(N_CORES)))
    return np.stack([res.results[b]["out"] for b in range(B)]).astype(np.float32)



# revision 4
# speedup vs baseline: 1.0058x; 1.0058x over previous
"""Causal self-attention (GPT-style block) on 8 Trainium2 NeuronCores.

Sharding: pure data-parallel over batch (B=8 -> 8 cores, no collectives).

v2: the QKV projection runs in fp8e4m3 DoubleRow matmuls with hi/lo
error compensation (3-product scheme), cutting its PE cost from 6 to 4.5
cycles/column while matching bf16 accuracy:
  x ~= xh + xl (both e4m3), w*32 ~= wh + wl (both e4m3, x32 scale keeps
  w out of the e4m3 subnormal range)
  x@w ~= (xh+xl)@wh + xh@wl
Each DoubleRow instruction contracts 2 k-tiles at 0.5 cycles/row:
  per c-chunk: DR[(xh_c,wh_c),(xl_c,wh_c)]; per chunk pair: DR[(xh_c,wl_c),
  (xh_c',wl_c')]. The wh operand is fed twice via a stride-0 tile dim
  (stationary) or a host-duplicated layout (moving side, for v).
Scale bookkeeping: q,k,v land at 32x; exp folds 1/(sqrt(hd)*32*32) into
its scale; the PV normalization folds 1/32 into the reciprocal multiply.

Attention (QK, exp, PV) and the output projection stay bf16 exactly as
the baseline: fp8 P (softmax weights) measurably breaks the 2e-2 error
budget, and compensated-fp8 QK costs the same as bf16.

Weight/x DMA is spread over the SP/Pool/DVE queues (never ACT, which the
exp chain needs); v-projection matmuls are interleaved between q/k pieces
so PSUM drains overlap PE work.
"""

import sys
from contextlib import ExitStack

import numpy as np

if "/opt/trn_rl_repo" not in sys.path:
    sys.path.insert(0, "/opt/trn_rl_repo")

import concourse.bacc as bacc
import concourse.bass as bass
import concourse.tile as tile
from concourse import mybir
from concourse.masks import make_upper_triangular

F32 = mybir.dt.float32
BF16 = mybir.dt.bfloat16
FP8 = mybir.dt.float8e4
DR = mybir.MatmulPerfMode.DoubleRow

T = 1024
C = 768
H = 12
HD = C // H  # 64
N_CORES = 8
WS = 32.0  # weight scale (power of 2): keeps w_attn out of e4m3 subnormals


def build_attention_core(t=T, repeats=1):
    nc = bacc.Bacc(None, target_bir_lowering=False, debug=False)
    xhl_d = nc.declare_dram_parameter("xhl", [128, 6, 2, t], FP8, isOutput=False)
    xhh_d = nc.declare_dram_parameter("xhh", [128, 3, 2, t], FP8, isOutput=False)
    wh_d = nc.declare_dram_parameter("wh", [128, 6, 3 * C], FP8, isOutput=False)
    wl_d = nc.declare_dram_parameter("wl", [128, 3, 2, 3 * C], FP8, isOutput=False)
    wvh2_d = nc.declare_dram_parameter("wvh2", [128, 6, 2, C], FP8, isOutput=False)
    b_attn = nc.declare_dram_parameter("b_attn", [3 * C], F32, isOutput=False)
    wpT_d = nc.declare_dram_parameter("wpT", [C, C], BF16, isOutput=False)
    b_proj = nc.declare_dram_parameter("b_proj", [C], F32, isOutput=False)
    out = nc.declare_dram_parameter("out", [t, C], F32, isOutput=True)

    with ExitStack() as octx:
        tc = octx.enter_context(tile.TileContext(nc))
        for _rep in range(repeats):
            _emit_once(nc, tc, t, xhl_d, xhh_d, wh_d, wl_d, wvh2_d, b_attn,
                       wpT_d, b_proj, out)
    nc.compile()
    return nc


def _emit_once(nc, tc, t, xhl_d, xhh_d, wh_d, wl_d, wvh2_d, b_attn, wpT_d,
               b_proj, out):
    NT = t // 128
    NCC = C // 128  # 6
    NHP = H // 2  # 6

    with ExitStack() as ctx:
        singles = ctx.enter_context(tc.tile_pool(name="singles", bufs=1))
        psum = ctx.enter_context(tc.tile_pool(name="psum", bufs=1, space="PSUM"))

        tri = singles.tile([128, 128], BF16)
        make_upper_triangular(nc, tri, val=1.0, diag=True)

        # b_attn[0:2C] * WS, one [128,1] column per qk o-chunk
        bias_qk = singles.tile([128, 2 * NCC], F32)
        nc.sync.dma_start(
            out=bias_qk,
            in_=b_attn[0 : 2 * C].rearrange("(c p) -> p c", p=128),
        )
        bias_qks = singles.tile([128, 2 * NCC], F32)
        nc.vector.tensor_scalar_mul(bias_qks, bias_qk, WS)

        bias_v = singles.tile([128, C], F32)
        bav = b_attn[2 * C : 3 * C].rearrange("(o c) -> o c", o=1)
        nc.gpsimd.dma_start(
            out=bias_v,
            in_=bass.AP(tensor=bav.tensor, offset=bav.offset, ap=[[0, 128]] + bav.ap[1:]),
        )
        bias_vs = singles.tile([128, C], F32)
        nc.vector.tensor_scalar_mul(bias_vs, bias_v, WS)

        bias_p = singles.tile([128, C], F32)
        bpv = b_proj[:].rearrange("(o c) -> o c", o=1)
        nc.gpsimd.dma_start(
            out=bias_p,
            in_=bass.AP(tensor=bpv.tensor, offset=bpv.offset, ap=[[0, 128]] + bpv.ap[1:]),
        )

        wpT = singles.tile([128, NCC, C], BF16, name="wpT")
        for cc in range(NCC):
            nc.gpsimd.dma_start(
                out=wpT[:, cc, :], in_=wpT_d[cc * 128 : (cc + 1) * 128, :]
            )

        def n_pieces(total, maxw=512):
            res = []
            s = 0
            while s < total:
                w = min(maxw, total - s)
                res.append((s, w))
                s += w
            return res

        pool_y = ctx.enter_context(tc.tile_pool(name="pool_y", bufs=1))
        y_nat = [pool_y.tile([128, C], BF16, name=f"ynat{j}") for j in range(NT)]

        pool2_cm = tc.tile_pool(name="pool_qkv", bufs=1)
        pool2 = pool2_cm.__enter__()

        qT = [pool2.tile([128, t], BF16, name=f"qT{j}") for j in range(NHP)]
        kT = [pool2.tile([128, t], BF16, name=f"kT{j}") for j in range(NHP)]
        v_aug = [pool2.tile([128, H, HD + 1], BF16, name=f"vaug{i}") for i in range(NT)]

        # ================= phase A+B: load + qkv (fp8 DoubleRow) ========
        with tc.tile_pool(name="pool1", bufs=1) as pool1:
            xhl = pool1.tile([128, NCC, 2, t], FP8, name="xhl")
            xhh = pool1.tile([128, 3, 2, t], FP8, name="xhh")
            for cc in range(NCC):
                nc.sync.dma_start(out=xhl[:, cc, :, :], in_=xhl_d[:, cc, :, :])
            for m in range(3):
                nc.sync.dma_start(out=xhh[:, m, :, :], in_=xhh_d[:, m, :, :])

            for i in range(NT):
                nc.vector.memset(v_aug[i][:, :, HD : HD + 1], 1.0)

            wh = pool1.tile([128, NCC, 3 * C], FP8, name="wh")
            wl = pool1.tile([128, 3, 2, 3 * C], FP8, name="wl")
            wvh2 = pool1.tile([128, NCC, 2, C], FP8, name="wvh2")
            for cc in range(NCC):
                nc.gpsimd.dma_start(out=wh[:, cc, :], in_=wh_d[:, cc, :])
            for m in range(3):
                nc.scalar.dma_start(out=wl[:, m, :, :], in_=wl_d[:, m, :, :])
            for cc in range(NCC):
                nc.gpsimd.dma_start(out=wvh2[:, cc, :, :], in_=wvh2_d[:, cc, :, :])

            def wh_dup(cc, o0, width):
                """wh[:, cc, o0:o0+width] with a stride-0 dim1 -> [128,2,width]."""
                base = wh[:, cc, o0 : o0 + width]
                return bass.AP(
                    tensor=base.tensor,
                    offset=base.offset,
                    ap=[base.ap[0], [0, 2], base.ap[-1]],
                )

            def emit_qk_piece(og, s, w):
                pq = psum.tile([128, 512], F32, name="ps_mm", tag="ps_mm", bufs=4)
                for cc in range(NCC):
                    nc.tensor.matmul(
                        pq[:, :w],
                        wh_dup(cc, og * 128, 128),
                        xhl[:, cc, :, s : s + w],
                        start=(cc == 0),
                        stop=False,
                        perf_mode=DR,
                    )
                for m in range(3):
                    nc.tensor.matmul(
                        pq[:, :w],
                        wl[:, m, :, og * 128 : (og + 1) * 128],
                        xhh[:, m, :, s : s + w],
                        start=False,
                        stop=(m == 2),
                        perf_mode=DR,
                    )
                dst = qT[og] if og < NCC else kT[og - NCC]
                nc.vector.tensor_scalar_add(
                    dst[:, s : s + w], pq[:, :w], bias_qks[:, og : og + 1]
                )

            def emit_v_chunk(p, it):
                pv = psum.tile([128, 128], F32, name="ps_v", tag="ps_v", bufs=2)
                for cc in range(NCC):
                    nc.tensor.matmul(
                        pv,
                        xhl[:, cc, :, it * 128 : (it + 1) * 128],
                        wvh2[:, cc, :, p * 128 : (p + 1) * 128],
                        start=(cc == 0),
                        stop=False,
                        perf_mode=DR,
                    )
                for m in range(3):
                    nc.tensor.matmul(
                        pv,
                        xhh[:, m, :, it * 128 : (it + 1) * 128],
                        wl[:, m, :, 2 * C + p * 128 : 2 * C + (p + 1) * 128],
                        start=False,
                        stop=(m == 2),
                        perf_mode=DR,
                    )
                nc.vector.tensor_add(
                    v_aug[it][:, 2 * p : 2 * p + 2, 0:HD],
                    pv.rearrange("p (h d) -> p h d", d=HD),
                    bias_vs[:, 128 * p : 128 * (p + 1)].rearrange(
                        "p (h d) -> p h d", d=HD
                    ),
                )

            def emit_qkv_pair(p):
                # interleave v chunks between the big q/k pieces so the
                # 2-buf ps_v drains hide behind PE work
                pieces = [(og, s, w) for og in (p, NCC + p) for (s, w) in n_pieces(t)]
                vi = 0
                for (og, s, w) in pieces:
                    emit_qk_piece(og, s, w)
                    for _ in range(2):
                        if vi < NT:
                            emit_v_chunk(p, vi)
                            vi += 1
                while vi < NT:
                    emit_v_chunk(p, vi)
                    vi += 1

            # ===== phase C: attention =====
            def emit_attention_qk(hp):
                hA, hB = 2 * hp, 2 * hp + 1
                eA = [
                    pool2.tile([128, t], BF16, name=f"eA{i}", tag=f"eA{i}", bufs=2)
                    for i in range(NT)
                ]
                eB = [
                    pool2.tile([128, t], BF16, name=f"eB{i}", tag=f"eB{i}", bufs=2)
                    for i in range(NT)
                ]
                for i in range(NT):
                    for (s, w) in n_pieces(t - 128 * i):
                        tq0 = 128 * i + s
                        for head, half, e in ((hA, 0, eA), (hB, 64, eB)):
                            ps = psum.tile(
                                [128, 512], F32, name="ps_s", tag="ps_mm", bufs=4
                            )
                            nc.tensor.matmul(
                                ps[:, :w],
                                kT[hp][half : half + 64, 128 * i : 128 * (i + 1)],
                                qT[hp][half : half + 64, tq0 : tq0 + w],
                                start=True,
                                stop=True,
                            )
                            nc.scalar.activation(
                                e[i][:, tq0 : tq0 + w],
                                ps[:, :w],
                                mybir.ActivationFunctionType.Exp,
                                bias=0.0,
                                scale=1.0 / float(np.sqrt(HD) * WS * WS),
                            )
                    d0 = 128 * i
                    nc.vector.tensor_mul(
                        eA[i][:, d0 : d0 + 128], eA[i][:, d0 : d0 + 128], tri
                    )
                    nc.vector.tensor_mul(
                        eB[i][:, d0 : d0 + 128], eB[i][:, d0 : d0 + 128], tri
                    )
                return eA, eB

            def emit_attention_pv(hp, eA, eB):
                hA, hB = 2 * hp, 2 * hp + 1
                for head, e in ((hA, eA), (hB, eB)):
                    for j in range(NT):
                        py = psum.tile([128, HD + 1], F32, name="ps_y", tag="ps_y", bufs=2)
                        for i in range(j + 1):
                            nc.tensor.matmul(
                                py,
                                e[i][:, 128 * j : 128 * (j + 1)],
                                v_aug[i][:, head, :],
                                start=(i == 0),
                                stop=(i == j),
                            )
                        rcp = pool2.tile([128, 1], F32, name="rcp", tag="rcp", bufs=4)
                        nc.vector.reciprocal(rcp, py[:, HD : HD + 1])
                        # y = py * rcp / WS  (v carries the x32 weight scale)
                        nc.vector.tensor_scalar(
                            y_nat[j][:, head * HD : (head + 1) * HD],
                            py[:, 0:HD],
                            rcp,
                            1.0 / WS,
                            op0=mybir.AluOpType.mult,
                            op1=mybir.AluOpType.mult,
                        )

            es = {}
            emit_qkv_pair(0)
            emit_qkv_pair(1)
            es[0] = emit_attention_qk(0)
            for p in range(2, NHP):
                emit_qkv_pair(p)
                es[p - 1] = emit_attention_qk(p - 1)
                emit_attention_pv(p - 2, *es.pop(p - 2))
            es[NHP - 1] = emit_attention_qk(NHP - 1)
            emit_attention_pv(NHP - 2, *es.pop(NHP - 2))
            emit_attention_pv(NHP - 1, *es.pop(NHP - 1))

        pool2_cm.__exit__(None, None, None)

        # ================= phase D+E: transpose y, project =============
        pool4 = ctx.enter_context(tc.tile_pool(name="pool_de", bufs=1))
        yTall = pool4.tile([128, NCC, t], BF16, name="yTall")
        for j in range(NT):
            nc.sync.dma_start_transpose(
                yTall[:, :, j * 128 : (j + 1) * 128], y_nat[j]
            )

        for it in range(NT):
            out_sb = pool4.tile([128, C], F32, name="out_sb", bufs=3)
            for (s, w) in n_pieces(C):
                po = psum.tile([128, 512], F32, name="ps_o", tag="ps_mm", bufs=4)
                for cc in range(NCC):
                    nc.tensor.matmul(
                        po[:, :w],
                        yTall[:, cc, it * 128 : (it + 1) * 128],
                        wpT[:, cc, s : s + w],
                        start=(cc == 0),
                        stop=(cc == NCC - 1),
                    )
                nc.vector.tensor_add(
                    out_sb[:, s : s + w], po[:, :w], bias_p[:, s : s + w]
                )
            eng = nc.gpsimd if it % 2 == 0 else nc.sync
            eng.dma_start(out=out[it * 128 : (it + 1) * 128, :], in_=out_sb)


_NC_CACHE = {}


def get_nc(t=T):
    if t not in _NC_CACHE:
        _NC_CACHE[t] = build_attention_core(t)
    return _NC_CACHE[t]


def host_prep(inputs):
    import ml_dtypes

    E4M3 = ml_dtypes.float8_e4m3
    BF = ml_dtypes.bfloat16

    x = np.asarray(inputs["x"], dtype=np.float32)
    b_attn = np.ascontiguousarray(inputs["b_attn"], dtype=np.float32)
    b_proj = np.ascontiguousarray(inputs["b_proj"], dtype=np.float32)

    waT = np.asarray(inputs["w_attn"], dtype=np.float32).T * WS  # [C, 3C]
    whf = waT.astype(E4M3)
    wlf = (waT - whf.astype(np.float32)).astype(E4M3)
    # wh: [128, 6, 3C]
    wh = np.ascontiguousarray(whf.reshape(6, 128, 3 * C).transpose(1, 0, 2))
    # wl: [128, 3, 2, 3C] chunk pairs
    wl = np.ascontiguousarray(wlf.reshape(3, 2, 128, 3 * C).transpose(2, 0, 1, 3))
    # wvh2: [128, 6, 2, C] = wv-hi duplicated on dim2
    wvh = whf[:, 2 * C :].reshape(6, 128, C).transpose(1, 0, 2)
    wvh2 = np.ascontiguousarray(
        np.broadcast_to(wvh[:, :, None, :], (128, 6, 2, C))
    )
    wpT = np.ascontiguousarray(
        np.asarray(inputs["w_proj"], dtype=np.float32).T
    ).astype(BF)

    per_core = []
    for b in range(x.shape[0]):
        xT = np.ascontiguousarray(x[b].T)  # [C, T]
        xh = xT.astype(E4M3)
        xl = (xT - xh.astype(np.float32)).astype(E4M3)
        # xhl: [128, 6, 2, T]
        xhl = np.empty((128, 6, 2, xT.shape[1]), dtype=E4M3)
        xhl[:, :, 0, :] = xh.reshape(6, 128, -1).transpose(1, 0, 2)
        xhl[:, :, 1, :] = xl.reshape(6, 128, -1).transpose(1, 0, 2)
        # xhh: [128, 3, 2, T] = xh chunk pairs
        xhh = np.ascontiguousarray(
            xh.reshape(3, 2, 128, -1).transpose(2, 0, 1, 3)
        )
        per_core.append(
            {
                "xhl": xhl,
                "xhh": xhh,
                "wh": wh,
                "wl": wl,
                "wvh2": wvh2,
                "b_attn": b_attn,
                "wpT": wpT,
                "b_proj": b_proj,
            }
        )
    return per_core


def kernel(**inputs):
    from concourse.bass_utils import run_bass_kernel_spmd

    x = inputs["x"]
    B, t, _ = x.shape
    assert B == N_CORES
    in_maps = host_prep(inputs)
    nc = get_nc(t)
    res = run_bass_kernel_spmd(nc, in_maps, core_ids=list(range(N_CORES)))
    return np.stack([res.results[b]["out"] for b in range(B)]).astype(np.float32)


# revision 6
# speedup vs baseline: 1.0288x; 1.0229x over previous
"""Causal self-attention (GPT-style block) on 8 Trainium2 NeuronCores.

Sharding: pure data-parallel over batch (B=8 -> 8 cores, no collectives).

v2: the QKV projection runs in fp8e4m3 DoubleRow matmuls with hi/lo
error compensation (3-product scheme), cutting its PE cost from 6 to 4.5
cycles/column while matching bf16 accuracy:
  x ~= xh + xl (both e4m3), w*32 ~= wh + wl (both e4m3, x32 scale keeps
  w out of the e4m3 subnormal range)
  x@w ~= (xh+xl)@wh + xh@wl
Each DoubleRow instruction contracts 2 k-tiles at 0.5 cycles/row:
  per c-chunk: DR[(xh_c,wh_c),(xl_c,wh_c)]; per chunk pair: DR[(xh_c,wl_c),
  (xh_c',wl_c')]. The wh operand is fed twice via a stride-0 tile dim
  (stationary) or a host-duplicated layout (moving side, for v).
Scale bookkeeping: q,k,v land at 32x; exp folds 1/(sqrt(hd)*32*32) into
its scale; the PV normalization folds 1/32 into the reciprocal multiply.

Attention (QK, exp, PV) and the output projection stay bf16 exactly as
the baseline: fp8 P (softmax weights) measurably breaks the 2e-2 error
budget, and compensated-fp8 QK costs the same as bf16.

Weight/x DMA is spread over the SP/Pool/DVE queues (never ACT, which the
exp chain needs); v-projection matmuls are interleaved between q/k pieces
so PSUM drains overlap PE work.
"""

import sys
from contextlib import ExitStack

import numpy as np

if "/opt/trn_rl_repo" not in sys.path:
    sys.path.insert(0, "/opt/trn_rl_repo")

import concourse.bacc as bacc
import concourse.bass as bass
import concourse.tile as tile
from concourse import mybir
from concourse.masks import make_upper_triangular

F32 = mybir.dt.float32
BF16 = mybir.dt.bfloat16
FP8 = mybir.dt.float8e4
DR = mybir.MatmulPerfMode.DoubleRow

T = 1024
C = 768
H = 12
HD = C // H  # 64
N_CORES = 8
WS = 32.0  # weight scale (power of 2): keeps w_attn out of e4m3 subnormals


def build_attention_core(t=T, repeats=1):
    nc = bacc.Bacc(None, target_bir_lowering=False, debug=False)
    xhl_d = nc.declare_dram_parameter("xhl", [128, 6, 2, t], FP8, isOutput=False)
    xhh_d = nc.declare_dram_parameter("xhh", [128, 3, 2, t], FP8, isOutput=False)
    wh_d = nc.declare_dram_parameter("wh", [128, 6, 3 * C], FP8, isOutput=False)
    wl_d = nc.declare_dram_parameter("wl", [128, 3, 2, 3 * C], FP8, isOutput=False)
    wvh2_d = nc.declare_dram_parameter("wvh2", [128, 6, 2, C], FP8, isOutput=False)
    b_attn = nc.declare_dram_parameter("b_attn", [3 * C], F32, isOutput=False)
    wpT_d = nc.declare_dram_parameter("wpT", [C, C], BF16, isOutput=False)
    b_proj = nc.declare_dram_parameter("b_proj", [C], F32, isOutput=False)
    out = nc.declare_dram_parameter("out", [t, C], F32, isOutput=True)

    with ExitStack() as octx:
        tc = octx.enter_context(tile.TileContext(nc))
        for _rep in range(repeats):
            _emit_once(nc, tc, t, xhl_d, xhh_d, wh_d, wl_d, wvh2_d, b_attn,
                       wpT_d, b_proj, out)
    nc.compile()
    return nc


def _emit_once(nc, tc, t, xhl_d, xhh_d, wh_d, wl_d, wvh2_d, b_attn, wpT_d,
               b_proj, out):
    NT = t // 128
    NCC = C // 128  # 6
    NHP = H // 2  # 6

    with ExitStack() as ctx:
        singles = ctx.enter_context(tc.tile_pool(name="singles", bufs=1))
        psum = ctx.enter_context(tc.tile_pool(name="psum", bufs=1, space="PSUM"))

        tri = singles.tile([128, 128], BF16)
        make_upper_triangular(nc, tri, val=1.0, diag=True)

        # b_attn[0:2C] * WS, one [128,1] column per qk o-chunk
        bias_qk = singles.tile([128, 2 * NCC], F32)
        nc.sync.dma_start(
            out=bias_qk,
            in_=b_attn[0 : 2 * C].rearrange("(c p) -> p c", p=128),
        )
        bias_qks = singles.tile([128, 2 * NCC], F32)
        nc.vector.tensor_scalar_mul(bias_qks, bias_qk, WS)

        # (loaded on ACT's queue: only needed from the first v-drain /
        # phase E on, and ACT is idle until the first attention pair)
        bias_v = singles.tile([128, C], F32)
        bav = b_attn[2 * C : 3 * C].rearrange("(o c) -> o c", o=1)
        nc.scalar.dma_start(
            out=bias_v,
            in_=bass.AP(tensor=bav.tensor, offset=bav.offset, ap=[[0, 128]] + bav.ap[1:]),
        )
        bias_vs = singles.tile([128, C], F32)
        nc.vector.tensor_scalar_mul(bias_vs, bias_v, WS)

        bias_p = singles.tile([128, C], F32)
        bpv = b_proj[:].rearrange("(o c) -> o c", o=1)
        nc.scalar.dma_start(
            out=bias_p,
            in_=bass.AP(tensor=bpv.tensor, offset=bpv.offset, ap=[[0, 128]] + bpv.ap[1:]),
        )

        wpT = singles.tile([128, NCC, C], BF16, name="wpT")

        def n_pieces(total, maxw=512):
            res = []
            s = 0
            while s < total:
                w = min(maxw, total - s)
                res.append((s, w))
                s += w
            return res

        pool_y = ctx.enter_context(tc.tile_pool(name="pool_y", bufs=1))
        y_nat = [pool_y.tile([128, C], BF16, name=f"ynat{j}") for j in range(NT)]

        pool2_cm = tc.tile_pool(name="pool_qkv", bufs=1)
        pool2 = pool2_cm.__enter__()

        qT = [pool2.tile([128, t], BF16, name=f"qT{j}") for j in range(NHP)]
        kT = [pool2.tile([128, t], BF16, name=f"kT{j}") for j in range(NHP)]
        v_aug = [pool2.tile([128, H, HD + 1], BF16, name=f"vaug{i}") for i in range(NT)]

        # ================= phase A+B: load + qkv (fp8 DoubleRow) ========
        with tc.tile_pool(name="pool1", bufs=1) as pool1:
            xhl = pool1.tile([128, NCC, 2, t], FP8, name="xhl")
            xhh = pool1.tile([128, 3, 2, t], FP8, name="xhh")
            for cc in range(NCC):
                nc.sync.dma_start(out=xhl[:, cc, :, :], in_=xhl_d[:, cc, :, :])
            for m in range(3):
                nc.sync.dma_start(out=xhh[:, m, :, :], in_=xhh_d[:, m, :, :])

            for i in range(NT):
                nc.vector.memset(v_aug[i][:, :, HD : HD + 1], 1.0)

            wh = pool1.tile([128, NCC, 3 * C], FP8, name="wh")
            wl = pool1.tile([128, 3, 2, 3 * C], FP8, name="wl")
            wvh2 = pool1.tile([128, NCC, 2, C], FP8, name="wvh2")
            # wh chunks first (the very first QKV group blocks on wh[0]),
            # then wl (ACT queue, idle this early), then wvh2, then wpT.
            for cc in range(NCC):
                nc.gpsimd.dma_start(out=wh[:, cc, :], in_=wh_d[:, cc, :])
            for m in range(3):
                nc.scalar.dma_start(out=wl[:, m, :, :], in_=wl_d[:, m, :, :])
            for cc in range(NCC):
                nc.gpsimd.dma_start(out=wvh2[:, cc, :, :], in_=wvh2_d[:, cc, :, :])
            for cc in range(NCC):
                nc.gpsimd.dma_start(
                    out=wpT[:, cc, :], in_=wpT_d[cc * 128 : (cc + 1) * 128, :]
                )

            def wh_dup(cc, o0, width):
                """wh[:, cc, o0:o0+width] with a stride-0 dim1 -> [128,2,width]."""
                base = wh[:, cc, o0 : o0 + width]
                return bass.AP(
                    tensor=base.tensor,
                    offset=base.offset,
                    ap=[base.ap[0], [0, 2], base.ap[-1]],
                )

            def emit_qk_piece(og, s, w):
                pq = psum.tile([128, 512], F32, name="ps_mm", tag="ps_mm", bufs=4)
                for cc in range(NCC):
                    nc.tensor.matmul(
                        pq[:, :w],
                        wh_dup(cc, og * 128, 128),
                        xhl[:, cc, :, s : s + w],
                        start=(cc == 0),
                        stop=False,
                        perf_mode=DR,
                    )
                for m in range(3):
                    nc.tensor.matmul(
                        pq[:, :w],
                        wl[:, m, :, og * 128 : (og + 1) * 128],
                        xhh[:, m, :, s : s + w],
                        start=False,
                        stop=(m == 2),
                        perf_mode=DR,
                    )
                dst = qT[og] if og < NCC else kT[og - NCC]
                nc.vector.tensor_scalar_add(
                    dst[:, s : s + w], pq[:, :w], bias_qks[:, og : og + 1]
                )

            def emit_v_chunk(p, it):
                pv = psum.tile([128, 128], F32, name="ps_v", tag="ps_v", bufs=2)
                for cc in range(NCC):
                    nc.tensor.matmul(
                        pv,
                        xhl[:, cc, :, it * 128 : (it + 1) * 128],
                        wvh2[:, cc, :, p * 128 : (p + 1) * 128],
                        start=(cc == 0),
                        stop=False,
                        perf_mode=DR,
                    )
                for m in range(3):
                    nc.tensor.matmul(
                        pv,
                        xhh[:, m, :, it * 128 : (it + 1) * 128],
                        wl[:, m, :, 2 * C + p * 128 : 2 * C + (p + 1) * 128],
                        start=False,
                        stop=(m == 2),
                        perf_mode=DR,
                    )
                nc.vector.tensor_add(
                    v_aug[it][:, 2 * p : 2 * p + 2, 0:HD],
                    pv.rearrange("p (h d) -> p h d", d=HD),
                    bias_vs[:, 128 * p : 128 * (p + 1)].rearrange(
                        "p (h d) -> p h d", d=HD
                    ),
                )

            def emit_qkv_pair(p):
                # interleave v chunks between the big q/k pieces so the
                # 2-buf ps_v drains hide behind PE work
                pieces = [(og, s, w) for og in (p, NCC + p) for (s, w) in n_pieces(t)]
                vi = 0
                for (og, s, w) in pieces:
                    emit_qk_piece(og, s, w)
                    for _ in range(2):
                        if vi < NT:
                            emit_v_chunk(p, vi)
                            vi += 1
                while vi < NT:
                    emit_v_chunk(p, vi)
                    vi += 1

            # ===== phase C: attention =====
            def emit_attention_qk(hp):
                hA, hB = 2 * hp, 2 * hp + 1
                eA = [
                    pool2.tile([128, t], BF16, name=f"eA{i}", tag=f"eA{i}", bufs=2)
                    for i in range(NT)
                ]
                eB = [
                    pool2.tile([128, t], BF16, name=f"eB{i}", tag=f"eB{i}", bufs=2)
                    for i in range(NT)
                ]
                for i in range(NT):
                    for (s, w) in n_pieces(t - 128 * i):
                        tq0 = 128 * i + s
                        for head, half, e in ((hA, 0, eA), (hB, 64, eB)):
                            ps = psum.tile(
                                [128, 512], F32, name="ps_s", tag="ps_mm", bufs=4
                            )
                            nc.tensor.matmul(
                                ps[:, :w],
                                kT[hp][half : half + 64, 128 * i : 128 * (i + 1)],
                                qT[hp][half : half + 64, tq0 : tq0 + w],
                                start=True,
                                stop=True,
                            )
                            nc.scalar.activation(
                                e[i][:, tq0 : tq0 + w],
                                ps[:, :w],
                                mybir.ActivationFunctionType.Exp,
                                bias=0.0,
                                scale=1.0 / float(np.sqrt(HD) * WS * WS),
                            )
                    d0 = 128 * i
                    nc.vector.tensor_mul(
                        eA[i][:, d0 : d0 + 128], eA[i][:, d0 : d0 + 128], tri
                    )
                    nc.vector.tensor_mul(
                        eB[i][:, d0 : d0 + 128], eB[i][:, d0 : d0 + 128], tri
                    )
                return eA, eB

            def emit_attention_pv(hp, eA, eB):
                hA, hB = 2 * hp, 2 * hp + 1
                for head, e in ((hA, eA), (hB, eB)):
                    for j in range(NT):
                        py = psum.tile([128, HD + 1], F32, name="ps_y", tag="ps_y", bufs=2)
                        for i in range(j + 1):
                            nc.tensor.matmul(
                                py,
                                e[i][:, 128 * j : 128 * (j + 1)],
                                v_aug[i][:, head, :],
                                start=(i == 0),
                                stop=(i == j),
                            )
                        rcp = pool2.tile([128, 1], F32, name="rcp", tag="rcp", bufs=4)
                        nc.vector.reciprocal(rcp, py[:, HD : HD + 1])
                        # y = py * rcp / WS  (v carries the x32 weight scale)
                        nc.vector.tensor_scalar(
                            y_nat[j][:, head * HD : (head + 1) * HD],
                            py[:, 0:HD],
                            rcp,
                            1.0 / WS,
                            op0=mybir.AluOpType.mult,
                            op1=mybir.AluOpType.mult,
                        )

            es = {}
            emit_qkv_pair(0)
            emit_qkv_pair(1)
            es[0] = emit_attention_qk(0)
            for p in range(2, NHP):
                emit_qkv_pair(p)
                es[p - 1] = emit_attention_qk(p - 1)
                emit_attention_pv(p - 2, *es.pop(p - 2))
            es[NHP - 1] = emit_attention_qk(NHP - 1)
            emit_attention_pv(NHP - 2, *es.pop(NHP - 2))
            emit_attention_pv(NHP - 1, *es.pop(NHP - 1))

        pool2_cm.__exit__(None, None, None)

        # ================= phase D+E: transpose y, project =============
        pool4 = ctx.enter_context(tc.tile_pool(name="pool_de", bufs=1))
        yTall = pool4.tile([128, NCC, t], BF16, name="yTall")
        for j in range(NT):
            nc.sync.dma_start_transpose(
                yTall[:, :, j * 128 : (j + 1) * 128], y_nat[j]
            )

        for it in range(NT):
            out_sb = pool4.tile([128, C], F32, name="out_sb", bufs=3)
            for (s, w) in n_pieces(C):
                po = psum.tile([128, 512], F32, name="ps_o", tag="ps_mm", bufs=4)
                for cc in range(NCC):
                    nc.tensor.matmul(
                        po[:, :w],
                        yTall[:, cc, it * 128 : (it + 1) * 128],
                        wpT[:, cc, s : s + w],
                        start=(cc == 0),
                        stop=(cc == NCC - 1),
                    )
                nc.vector.tensor_add(
                    out_sb[:, s : s + w], po[:, :w], bias_p[:, s : s + w]
                )
            eng = nc.gpsimd if it % 2 == 0 else nc.sync
            eng.dma_start(out=out[it * 128 : (it + 1) * 128, :], in_=out_sb)


_NC_CACHE = {}


def get_nc(t=T):
    if t not in _NC_CACHE:
        _NC_CACHE[t] = build_attention_core(t)
    return _NC_CACHE[t]


def host_prep(inputs):
    import ml_dtypes

    E4M3 = ml_dtypes.float8_e4m3
    BF = ml_dtypes.bfloat16

    x = np.asarray(inputs["x"], dtype=np.float32)
    b_attn = np.ascontiguousarray(inputs["b_attn"], dtype=np.float32)
    b_proj = np.ascontiguousarray(inputs["b_proj"], dtype=np.float32)

    waT = np.asarray(inputs["w_attn"], dtype=np.float32).T * WS  # [C, 3C]
    whf = waT.astype(E4M3)
    wlf = (waT - whf.astype(np.float32)).astype(E4M3)
    # wh: [128, 6, 3C]
    wh = np.ascontiguousarray(whf.reshape(6, 128, 3 * C).transpose(1, 0, 2))
    # wl: [128, 3, 2, 3C] chunk pairs
    wl = np.ascontiguousarray(wlf.reshape(3, 2, 128, 3 * C).transpose(2, 0, 1, 3))
    # wvh2: [128, 6, 2, C] = wv-hi duplicated on dim2
    wvh = whf[:, 2 * C :].reshape(6, 128, C).transpose(1, 0, 2)
    wvh2 = np.ascontiguousarray(
        np.broadcast_to(wvh[:, :, None, :], (128, 6, 2, C))
    )
    wpT = np.ascontiguousarray(
        np.asarray(inputs["w_proj"], dtype=np.float32).T
    ).astype(BF)

    per_core = []
    for b in range(x.shape[0]):
        xT = np.ascontiguousarray(x[b].T)  # [C, T]
        xh = xT.astype(E4M3)
        xl = (xT - xh.astype(np.float32)).astype(E4M3)
        # xhl: [128, 6, 2, T]
        xhl = np.empty((128, 6, 2, xT.shape[1]), dtype=E4M3)
        xhl[:, :, 0, :] = xh.reshape(6, 128, -1).transpose(1, 0, 2)
        xhl[:, :, 1, :] = xl.reshape(6, 128, -1).transpose(1, 0, 2)
        # xhh: [128, 3, 2, T] = xh chunk pairs
        xhh = np.ascontiguousarray(
            xh.reshape(3, 2, 128, -1).transpose(2, 0, 1, 3)
        )
        per_core.append(
            {
                "xhl": xhl,
                "xhh": xhh,
                "wh": wh,
                "wl": wl,
                "wvh2": wvh2,
                "b_attn": b_attn,
                "wpT": wpT,
                "b_proj": b_proj,
            }
        )
    return per_core


def kernel(**inputs):
    from concourse.bass_utils import run_bass_kernel_spmd

    x = inputs["x"]
    B, t, _ = x.shape
    assert B == N_CORES
    in_maps = host_prep(inputs)
    nc = get_nc(t)
    res = run_bass_kernel_spmd(nc, in_maps, core_ids=list(range(N_CORES)))
    return np.stack([res.results[b]["out"] for b in range(B)]).astype(np.float32)


# revision 9
# speedup vs baseline: 1.0440x; 1.0147x over previous
"""Causal self-attention (GPT-style block) on 8 Trainium2 NeuronCores.

Sharding: pure data-parallel over batch (B=8 -> 8 cores, no collectives).

v2: the QKV projection runs in fp8e4m3 DoubleRow matmuls with hi/lo
error compensation (3-product scheme), cutting its PE cost from 6 to 4.5
cycles/column while matching bf16 accuracy:
  x ~= xh + xl (both e4m3), w*32 ~= wh + wl (both e4m3, x32 scale keeps
  w out of the e4m3 subnormal range)
  x@w ~= (xh+xl)@wh + xh@wl
Each DoubleRow instruction contracts 2 k-tiles at 0.5 cycles/row:
  per c-chunk: DR[(xh_c,wh_c),(xl_c,wh_c)]; per chunk pair: DR[(xh_c,wl_c),
  (xh_c',wl_c')]. The wh operand is fed twice via a stride-0 tile dim
  (stationary) or a host-duplicated layout (moving side, for v).
Scale bookkeeping: q,k,v land at 32x; exp folds 1/(sqrt(hd)*32*32) into
its scale; the PV normalization folds 1/32 into the reciprocal multiply.

Attention (QK, exp, PV) and the output projection stay bf16 exactly as
the baseline: fp8 P (softmax weights) measurably breaks the 2e-2 error
budget, and compensated-fp8 QK costs the same as bf16.

Weight/x DMA is spread over the SP/Pool/DVE queues (never ACT, which the
exp chain needs); v-projection matmuls are interleaved between q/k pieces
so PSUM drains overlap PE work.
"""

import sys
from contextlib import ExitStack

import numpy as np

if "/opt/trn_rl_repo" not in sys.path:
    sys.path.insert(0, "/opt/trn_rl_repo")

import concourse.bacc as bacc
import concourse.bass as bass
import concourse.tile as tile
from concourse import mybir
from concourse.masks import make_upper_triangular

F32 = mybir.dt.float32
BF16 = mybir.dt.bfloat16
FP8 = mybir.dt.float8e4
DR = mybir.MatmulPerfMode.DoubleRow

T = 1024
C = 768
H = 12
HD = C // H  # 64
N_CORES = 8
WS = 32.0  # weight scale (power of 2): keeps w_attn out of e4m3 subnormals


def build_attention_core(t=T, repeats=1):
    nc = bacc.Bacc(None, target_bir_lowering=False, debug=False)
    xhl_d = nc.declare_dram_parameter("xhl", [128, 6, 2, t], FP8, isOutput=False)
    xhh_d = nc.declare_dram_parameter("xhh", [128, 3, 2, t], FP8, isOutput=False)
    wh_d = nc.declare_dram_parameter("wh", [128, 6, 3 * C], FP8, isOutput=False)
    wl_d = nc.declare_dram_parameter("wl", [128, 3, 2, 3 * C], FP8, isOutput=False)
    wvh2_d = nc.declare_dram_parameter("wvh2", [128, 6, 2, C], FP8, isOutput=False)
    b_attn = nc.declare_dram_parameter("b_attn", [3 * C], F32, isOutput=False)
    wpT_d = nc.declare_dram_parameter("wpT", [C, C], BF16, isOutput=False)
    b_proj = nc.declare_dram_parameter("b_proj", [C], F32, isOutput=False)
    out = nc.declare_dram_parameter("out", [t, C], F32, isOutput=True)

    with ExitStack() as octx:
        tc = octx.enter_context(tile.TileContext(nc))
        for _rep in range(repeats):
            _emit_once(nc, tc, t, xhl_d, xhh_d, wh_d, wl_d, wvh2_d, b_attn,
                       wpT_d, b_proj, out)
    nc.compile()
    return nc


def _emit_once(nc, tc, t, xhl_d, xhh_d, wh_d, wl_d, wvh2_d, b_attn, wpT_d,
               b_proj, out):
    NT = t // 128
    NCC = C // 128  # 6
    NHP = H // 2  # 6

    with ExitStack() as ctx:
        singles = ctx.enter_context(tc.tile_pool(name="singles", bufs=1))
        psum = ctx.enter_context(tc.tile_pool(name="psum", bufs=1, space="PSUM"))

        tri = singles.tile([128, 128], BF16)
        make_upper_triangular(nc, tri, val=1.0, diag=True)

        # b_attn[0:2C] * WS, one [128,1] column per qk o-chunk
        bias_qk = singles.tile([128, 2 * NCC], F32)
        nc.sync.dma_start(
            out=bias_qk,
            in_=b_attn[0 : 2 * C].rearrange("(c p) -> p c", p=128),
        )
        bias_qks = singles.tile([128, 2 * NCC], F32)
        nc.vector.tensor_scalar_mul(bias_qks, bias_qk, WS)

        # (loaded on ACT's queue: only needed from the first v-drain /
        # phase E on, and ACT is idle until the first attention pair)
        bias_v = singles.tile([128, C], F32)
        bav = b_attn[2 * C : 3 * C].rearrange("(o c) -> o c", o=1)
        nc.scalar.dma_start(
            out=bias_v,
            in_=bass.AP(tensor=bav.tensor, offset=bav.offset, ap=[[0, 128]] + bav.ap[1:]),
        )
        bias_vs = singles.tile([128, C], F32)
        nc.vector.tensor_scalar_mul(bias_vs, bias_v, WS)

        bias_p = singles.tile([128, C], F32)
        bpv = b_proj[:].rearrange("(o c) -> o c", o=1)
        nc.scalar.dma_start(
            out=bias_p,
            in_=bass.AP(tensor=bpv.tensor, offset=bpv.offset, ap=[[0, 128]] + bpv.ap[1:]),
        )

        wpT = singles.tile([128, NCC, C], BF16, name="wpT")

        def n_pieces(total, maxw=512):
            res = []
            s = 0
            while s < total:
                w = min(maxw, total - s)
                res.append((s, w))
                s += w
            return res

        pool_y = ctx.enter_context(tc.tile_pool(name="pool_y", bufs=1))
        y_nat = [pool_y.tile([128, C], BF16, name=f"ynat{j}") for j in range(NT)]

        pool2 = ctx.enter_context(tc.tile_pool(name="pool_qkv", bufs=1))

        qT = [pool2.tile([128, t], BF16, name=f"qT{j}") for j in range(NHP)]
        kT = [pool2.tile([128, t], BF16, name=f"kT{j}") for j in range(NHP)]
        v_aug = [pool2.tile([128, H, HD + 1], BF16, name=f"vaug{i}") for i in range(NT)]

        # ================= phase A+B: load + qkv (fp8 DoubleRow) ========
        with tc.tile_pool(name="pool1", bufs=1) as pool1:
            xhl = pool1.tile([128, NCC, 2, t], FP8, name="xhl")
            xhh = pool1.tile([128, 3, 2, t], FP8, name="xhh")
            for cc in range(NCC):
                nc.sync.dma_start(out=xhl[:, cc, :, :], in_=xhl_d[:, cc, :, :])
            for m in range(3):
                nc.sync.dma_start(out=xhh[:, m, :, :], in_=xhh_d[:, m, :, :])

            for i in range(NT):
                nc.vector.memset(v_aug[i][:, :, HD : HD + 1], 1.0)

            wh = pool1.tile([128, NCC, 3 * C], FP8, name="wh")
            wl = pool1.tile([128, 3, 2, 3 * C], FP8, name="wl")
            wvh2 = pool1.tile([128, NCC, 2, C], FP8, name="wvh2")
            # wh chunks first (the very first QKV group blocks on wh[0]),
            # then wl (ACT queue, idle this early), then wvh2, then wpT.
            for cc in range(NCC):
                nc.gpsimd.dma_start(out=wh[:, cc, :], in_=wh_d[:, cc, :])
            for m in range(3):
                nc.scalar.dma_start(out=wl[:, m, :, :], in_=wl_d[:, m, :, :])
            for cc in range(NCC):
                nc.gpsimd.dma_start(out=wvh2[:, cc, :, :], in_=wvh2_d[:, cc, :, :])
            for cc in range(NCC):
                nc.gpsimd.dma_start(
                    out=wpT[:, cc, :], in_=wpT_d[cc * 128 : (cc + 1) * 128, :]
                )

            def wh_dup(cc, o0, width):
                """wh[:, cc, o0:o0+width] with a stride-0 dim1 -> [128,2,width]."""
                base = wh[:, cc, o0 : o0 + width]
                return bass.AP(
                    tensor=base.tensor,
                    offset=base.offset,
                    ap=[base.ap[0], [0, 2], base.ap[-1]],
                )

            def emit_qk_piece(og, s, w):
                pq = psum.tile([128, 512], F32, name="ps_mm", tag="ps_mm", bufs=4)
                for cc in range(NCC):
                    nc.tensor.matmul(
                        pq[:, :w],
                        wh_dup(cc, og * 128, 128),
                        xhl[:, cc, :, s : s + w],
                        start=(cc == 0),
                        stop=False,
                        perf_mode=DR,
                    )
                for m in range(3):
                    nc.tensor.matmul(
                        pq[:, :w],
                        wl[:, m, :, og * 128 : (og + 1) * 128],
                        xhh[:, m, :, s : s + w],
                        start=False,
                        stop=(m == 2),
                        perf_mode=DR,
                    )
                dst = qT[og] if og < NCC else kT[og - NCC]
                nc.vector.tensor_scalar_add(
                    dst[:, s : s + w], pq[:, :w], bias_qks[:, og : og + 1]
                )

            def emit_v_chunk(p, it):
                pv = psum.tile([128, 128], F32, name="ps_v", tag="ps_v", bufs=2)
                for cc in range(NCC):
                    nc.tensor.matmul(
                        pv,
                        xhl[:, cc, :, it * 128 : (it + 1) * 128],
                        wvh2[:, cc, :, p * 128 : (p + 1) * 128],
                        start=(cc == 0),
                        stop=False,
                        perf_mode=DR,
                    )
                for m in range(3):
                    nc.tensor.matmul(
                        pv,
                        xhh[:, m, :, it * 128 : (it + 1) * 128],
                        wl[:, m, :, 2 * C + p * 128 : 2 * C + (p + 1) * 128],
                        start=False,
                        stop=(m == 2),
                        perf_mode=DR,
                    )
                nc.vector.tensor_add(
                    v_aug[it][:, 2 * p : 2 * p + 2, 0:HD],
                    pv.rearrange("p (h d) -> p h d", d=HD),
                    bias_vs[:, 128 * p : 128 * (p + 1)].rearrange(
                        "p (h d) -> p h d", d=HD
                    ),
                )

            def emit_qkv_pair(p):
                # interleave v chunks between the big q/k pieces so the
                # 2-buf ps_v drains hide behind PE work
                pieces = [(og, s, w) for og in (p, NCC + p) for (s, w) in n_pieces(t)]
                vi = 0
                for (og, s, w) in pieces:
                    emit_qk_piece(og, s, w)
                    # pair 0: defer v until after the q/k pieces — the v
                    # operands (wvh2, xhh) arrive late in the initial DMA
                    # stream and would stall the in-order PE
                    if p > 0:
                        for _ in range(2):
                            if vi < NT:
                                emit_v_chunk(p, vi)
                                vi += 1
                while vi < NT:
                    emit_v_chunk(p, vi)
                    vi += 1

            # ===== phase C: attention =====
            def emit_attention_qk(hp):
                hA, hB = 2 * hp, 2 * hp + 1
                eA = [
                    pool2.tile([128, t], BF16, name=f"eA{i}", tag=f"eA{i}", bufs=2)
                    for i in range(NT)
                ]
                eB = [
                    pool2.tile([128, t], BF16, name=f"eB{i}", tag=f"eB{i}", bufs=2)
                    for i in range(NT)
                ]
                for i in range(NT):
                    for (s, w) in n_pieces(t - 128 * i):
                        tq0 = 128 * i + s
                        for head, half, e in ((hA, 0, eA), (hB, 64, eB)):
                            ps = psum.tile(
                                [128, 512], F32, name="ps_s", tag="ps_mm", bufs=4
                            )
                            nc.tensor.matmul(
                                ps[:, :w],
                                kT[hp][half : half + 64, 128 * i : 128 * (i + 1)],
                                qT[hp][half : half + 64, tq0 : tq0 + w],
                                start=True,
                                stop=True,
                            )
                            nc.scalar.activation(
                                e[i][:, tq0 : tq0 + w],
                                ps[:, :w],
                                mybir.ActivationFunctionType.Exp,
                                bias=0.0,
                                scale=1.0 / float(np.sqrt(HD) * WS * WS),
                            )
                    d0 = 128 * i
                    nc.vector.tensor_mul(
                        eA[i][:, d0 : d0 + 128], eA[i][:, d0 : d0 + 128], tri
                    )
                    nc.vector.tensor_mul(
                        eB[i][:, d0 : d0 + 128], eB[i][:, d0 : d0 + 128], tri
                    )
                return eA, eB

            def emit_pv_head_chunk(e, head, j):
                py = psum.tile([128, HD + 1], F32, name="ps_y", tag="ps_y", bufs=2)
                for i in range(j + 1):
                    nc.tensor.matmul(
                        py,
                        e[i][:, 128 * j : 128 * (j + 1)],
                        v_aug[i][:, head, :],
                        start=(i == 0),
                        stop=(i == j),
                    )
                rcp = pool2.tile([128, 1], F32, name="rcp", tag="rcp", bufs=4)
                nc.vector.reciprocal(rcp, py[:, HD : HD + 1])
                # y = py * rcp / WS  (v carries the x32 weight scale)
                nc.vector.tensor_scalar(
                    y_nat[j][:, head * HD : (head + 1) * HD],
                    py[:, 0:HD],
                    rcp,
                    1.0 / WS,
                    op0=mybir.AluOpType.mult,
                    op1=mybir.AluOpType.mult,
                )

            def emit_attention_pv(hp, eA, eB):
                hA, hB = 2 * hp, 2 * hp + 1
                for head, e in ((hA, eA), (hB, eB)):
                    for j in range(NT):
                        emit_pv_head_chunk(e, head, j)

            es = {}
            emit_qkv_pair(0)
            emit_qkv_pair(1)
            es[0] = emit_attention_qk(0)
            for p in range(2, NHP):
                emit_qkv_pair(p)
                es[p - 1] = emit_attention_qk(p - 1)
                emit_attention_pv(p - 2, *es.pop(p - 2))
            es[NHP - 1] = emit_attention_qk(NHP - 1)
            emit_attention_pv(NHP - 2, *es.pop(NHP - 2))
            eA_fin, eB_fin = es.pop(NHP - 1)

        # ====== final PV pair, pipelined with transpose + projection ======
        # pool1 just closed; pool_de reuses its space while pool2 stays live.
        pool4 = ctx.enter_context(tc.tile_pool(name="pool_de", bufs=1))
        yTall = pool4.tile([128, NCC, t], BF16, name="yTall")

        def emit_proj(it):
            out_sb = pool4.tile([128, C], F32, name="out_sb", bufs=3)
            for (s, w) in n_pieces(C):
                po = psum.tile([128, 512], F32, name="ps_o", tag="ps_mm", bufs=4)
                for cc in range(NCC):
                    nc.tensor.matmul(
                        po[:, :w],
                        yTall[:, cc, it * 128 : (it + 1) * 128],
                        wpT[:, cc, s : s + w],
                        start=(cc == 0),
                        stop=(cc == NCC - 1),
                    )
                nc.vector.tensor_add(
                    out_sb[:, s : s + w], po[:, :w], bias_p[:, s : s + w]
                )
            eng = nc.gpsimd if it % 2 == 0 else nc.sync
            eng.dma_start(out=out[it * 128 : (it + 1) * 128, :], in_=out_sb)

        hA, hB = 2 * (NHP - 1), 2 * (NHP - 1) + 1
        for j in range(NT):
            emit_pv_head_chunk(eA_fin, hA, j)
            emit_pv_head_chunk(eB_fin, hB, j)
            nc.sync.dma_start_transpose(
                yTall[:, :, j * 128 : (j + 1) * 128], y_nat[j]
            )
            if j >= 1:
                emit_proj(j - 1)
        emit_proj(NT - 1)


_NC_CACHE = {}


def get_nc(t=T):
    if t not in _NC_CACHE:
        _NC_CACHE[t] = build_attention_core(t)
    return _NC_CACHE[t]


def host_prep(inputs):
    import ml_dtypes

    E4M3 = ml_dtypes.float8_e4m3
    BF = ml_dtypes.bfloat16

    x = np.asarray(inputs["x"], dtype=np.float32)
    b_attn = np.ascontiguousarray(inputs["b_attn"], dtype=np.float32)
    b_proj = np.ascontiguousarray(inputs["b_proj"], dtype=np.float32)

    waT = np.asarray(inputs["w_attn"], dtype=np.float32).T * WS  # [C, 3C]
    whf = waT.astype(E4M3)
    wlf = (waT - whf.astype(np.float32)).astype(E4M3)
    # wh: [128, 6, 3C]
    wh = np.ascontiguousarray(whf.reshape(6, 128, 3 * C).transpose(1, 0, 2))
    # wl: [128, 3, 2, 3C] chunk pairs
    wl = np.ascontiguousarray(wlf.reshape(3, 2, 128, 3 * C).transpose(2, 0, 1, 3))
    # wvh2: [128, 6, 2, C] = wv-hi duplicated on dim2
    wvh = whf[:, 2 * C :].reshape(6, 128, C).transpose(1, 0, 2)
    wvh2 = np.ascontiguousarray(
        np.broadcast_to(wvh[:, :, None, :], (128, 6, 2, C))
    )
    wpT = np.ascontiguousarray(
        np.asarray(inputs["w_proj"], dtype=np.float32).T
    ).astype(BF)

    per_core = []
    for b in range(x.shape[0]):
        xT = np.ascontiguousarray(x[b].T)  # [C, T]
        xh = xT.astype(E4M3)
        xl = (xT - xh.astype(np.float32)).astype(E4M3)
        # xhl: [128, 6, 2, T]
        xhl = np.empty((128, 6, 2, xT.shape[1]), dtype=E4M3)
        xhl[:, :, 0, :] = xh.reshape(6, 128, -1).transpose(1, 0, 2)
        xhl[:, :, 1, :] = xl.reshape(6, 128, -1).transpose(1, 0, 2)
        # xhh: [128, 3, 2, T] = xh chunk pairs
        xhh = np.ascontiguousarray(
            xh.reshape(3, 2, 128, -1).transpose(2, 0, 1, 3)
        )
        per_core.append(
            {
                "xhl": xhl,
                "xhh": xhh,
                "wh": wh,
                "wl": wl,
                "wvh2": wvh2,
                "b_attn": b_attn,
                "wpT": wpT,
                "b_proj": b_proj,
            }
        )
    return per_core


def kernel(**inputs):
    from concourse.bass_utils import run_bass_kernel_spmd

    x = inputs["x"]
    B, t, _ = x.shape
    assert B == N_CORES
    in_maps = host_prep(inputs)
    nc = get_nc(t)
    res = run_bass_kernel_spmd(nc, in_maps, core_ids=list(range(N_CORES)))
    return np.stack([res.results[b]["out"] for b in range(B)]).astype(np.float32)
